# revision 56
# baseline (speedup 1.0000x reference)
"""Chamfer p=5 loss (nn_ChamferLossP) — Bass kernel for 8x TRN2 NeuronCores.

Sharding: data-parallel over the batch dim B=8, one batch per core; host
combines the per-core partial sums (the final "mean all-reduce").

Per-core device algorithm (direction 1 shown; direction 2 swaps x<->y):

  argmin_m ||x_n - y_m||^2 == argmax_m key[n,m],  key = -||x_n - y_m||^2.

  The PE materialises key in PSUM slabs [128n x 2048m] with bf16 matmuls:
  each fp32 factor splits into 3 bf16 limbs and the significant limb
  products (plus limbs of -|y|^2 and -|x|^2) form a 24-term contraction,
  giving fp32-accurate keys equal to the NEGATED squared distance.
  Recentering to -d makes later fp16 rounding error scale with d itself,
  so selection stays sharp exactly where it matters (small d).

  The PSUM drain (the bottleneck) is split across two engines: ACT
  copies each slab to SBUF fp16 (1 elem/cyc @1.2GHz, the only other
  PSUM reader), and the DVE reduces each chunk's [128, 4096] fp16 row
  to 256 group maxima with a fused fold-by-halves chain (4x
  tensor_tensor max in 2x_1P mode: 4096->2048->1024->512->256), at
  ~0.65 ns/elem vs 1.1 for a direct 1x tensor_reduce from PSUM.
  Groups are the stride-256 residue classes {g + 256k}.

  Per chunk, nc.vector.max + max_index (the Max8/FindIndex8 HW) give
  the winning group id directly; one indirect DMA per 128-row chunk
  gathers that group's 16 candidate points (48 contiguous floats) from
  a host-built stride-256-ordered table.  The epilogue recomputes the
  16 exact fp32 squared distances, picks the winner (GpSimd does the
  power chain; |.|^5 via signed d^5 + apply_absolute_value on the
  reduce), and accumulates sum_c |x - nn|^5 into per-piece partials.
"""

import numpy as np
import ml_dtypes

import concourse.bass as bass
import concourse.bacc as bacc
import concourse.mybir as mybir
from concourse import bass_utils
from concourse.tile import TileContext

F32 = mybir.dt.float32
BF16 = mybir.dt.bfloat16
FP16 = mybir.dt.float16
I32 = mybir.dt.int32
U32 = mybir.dt.uint32
AF = mybir.ActivationFunctionType
ALU = mybir.AluOpType

B = 8
N_FULL = 4096
HALF_FULL = 2048
P = 128
R = 16              # argmin group size (candidates per gather)
KSPLIT = 24         # bf16 split-contraction terms (keys = -d, both norms)
MMFD = 512          # matmul free dim (PSUM-bank cap: N <= 512 fp32)
N_DIRECT = 0        # direct-drain chunks (0: fold-drain everything — measured best)


def _build_nc(N=N_FULL, HALF=HALF_FULL, num_devices=B):
    NCH = N // P         # 128-row chunks per direction
    NH = N // HALF       # psum slabs per chunk
    NGH = HALF // R      # groups per slab (128)
    NG = N // R          # groups per chunk row (256)

    nc = bacc.Bacc("TRN2", target_bir_lowering=False,
                   num_devices=num_devices)

    # augs columns: [x1_lhsT | y1_rhs | y2_lhsT | x2_rhs], each N wide.
    augs = nc.dram_tensor("augs", [KSPLIT, 4 * N], BF16,
                          kind="ExternalInput").ap()
    xr = nc.dram_tensor("xr", [N, 3], F32, kind="ExternalInput").ap()
    yr = nc.dram_tensor("yr", [N, 3], F32, kind="ExternalInput").ap()
    # gather tables, one per direction: [2*256, 48] (fold | direct)
    tbl1 = nc.dram_tensor("tbl1", [2 * NG, R * 3], F32,
                          kind="ExternalInput").ap()
    tbl2 = nc.dram_tensor("tbl2", [2 * NG, R * 3], F32,
                          kind="ExternalInput").ap()
    # consts row: [iota16 | iota16 + R]
    consts = nc.dram_tensor("consts", [P, 2 * R], F32,
                            kind="ExternalInput").ap()
    out_s = nc.dram_tensor("out_s", [P, 10], F32,
                           kind="ExternalOutput").ap()

    with TileContext(nc) as tc:
        with (
            tc.tile_pool(name="const", bufs=1) as const_pool,
            tc.tile_pool(name="eb", bufs=2) as eb_pool,
            tc.tile_pool(name="fold", bufs=2) as fold_pool,
            tc.tile_pool(name="u", bufs=4) as u_pool,
            tc.tile_pool(name="idx", bufs=4) as idx_pool,
            tc.tile_pool(name="epi", bufs=1) as epi_pool,
            tc.tile_pool(name="psum", bufs=2, space="PSUM") as psum_pool,
        ):
            augs_sb = const_pool.tile([KSPLIT, 4 * N], BF16, tag="augs")
            # load order: dir-1 lhsT, then dir-1 rhs in quarters (so the
            # first matmuls start as soon as the first piece lands);
            # lhsT on the SP ring, rhs on the ACT ring.
            nc.sync.dma_start(augs_sb[:, 0:P], augs[:, 0:P])
            for k in range(4):
                nc.scalar.dma_start(
                    augs_sb[:, N + k * (N // 4):N + (k + 1) * (N // 4)],
                    augs[:, N + k * (N // 4):N + (k + 1) * (N // 4)])
            nc.sync.dma_start(augs_sb[:, P:N], augs[:, P:N])
            for i in (2, 3):
                nc.sync.dma_start(augs_sb[:, i * N:(i + 1) * N],
                                  augs[:, i * N:(i + 1) * N])

            def aug(i):
                return augs_sb[:, i * N:(i + 1) * N]

            consts_sb = const_pool.tile([P, 2 * R], F32, tag="consts")
            nc.sync.dma_start(consts_sb[:], consts)

            # epilogue "own point" tiles
            ow_t = {}
            for dirn, own in ((1, xr), (2, yr)):
                ow = epi_pool.tile([P, NCH, 3], F32, tag=f"ow{dirn}",
                                   name=f"ow{dirn}")
                nc.sync.dma_start(
                    ow[:], own.rearrange("(c p) d -> p c d", p=P))
                ow_t[dirn] = ow

            # gathered candidate groups, flat [P, NCH * R * 3]
            cand = {1: epi_pool.tile([P, NCH * R * 3], F32, tag="cand1",
                                     name="cand1"),
                    2: epi_pool.tile([P, NCH * R * 3], F32, tag="cand2",
                                     name="cand2")}

            partials = epi_pool.tile([P, 10], F32, tag="partials")
            nc.vector.memset(partials[:], 0.0)

            def epilogue(dirn, half, c0, c1):
                """Exact within-group argmin + sum |diff|^5 for chunk range
                [c0, c1) of a direction; writes partials column
                (dirn-1)*3 + half."""
                NC_h = c1 - c0
                FCh = NC_h * R * 3
                FKh = NC_h * R
                hh = f"{dirn}_{half}"
                cd = cand[dirn][:, c0 * R * 3:c1 * R * 3]
                ow = ow_t[dirn]
                owb = bass.AP(ow[:].tensor, ow[:].offset + c0 * 3,
                              [ow[:].ap[0], [3, NC_h], [0, R], [1, 3]])

                diff = epi_pool.tile([P, FCh], F32, tag=f"df{hh}",
                                     name=f"df{hh}")
                nc.vector.tensor_sub(
                    diff[:].rearrange("p (c k d) -> p c k d", k=R, d=3),
                    owb, cd.rearrange("p (c k d) -> p c k d", k=R, d=3))
                # sq first (feeds the DVE distance chain); squares on
                # GpSimd (ACT is busy with the drain copies).  p5e is
                # SIGNED d^5 — the reduce below applies |.| per element.
                sq = epi_pool.tile([P, FCh], F32, tag=f"sq{hh}",
                                   name=f"sq{hh}")
                nc.gpsimd.tensor_mul(sq[:], diff[:], diff[:])
                q4 = epi_pool.tile([P, FCh], F32, tag=f"q4{hh}",
                                   name=f"q4{hh}")
                nc.gpsimd.tensor_mul(q4[:], sq[:], sq[:])
                p5e = epi_pool.tile([P, FCh], F32, tag=f"p5{hh}",
                                    name=f"p5{hh}")
                nc.gpsimd.tensor_mul(p5e[:], q4[:], diff[:])
                # squared L2 distance per candidate
                dd = epi_pool.tile([P, FKh], F32, tag=f"dd{hh}",
                                   name=f"dd{hh}")
                nc.vector.tensor_reduce(
                    out=dd[:], in_=sq[:].rearrange("p (k d) -> p k d", d=3),
                    axis=mybir.AxisListType.X, op=ALU.add)
                # min distance per row
                dmin = epi_pool.tile([P, NC_h], F32, tag=f"dm{hh}",
                                     name=f"dm{hh}")
                nc.vector.tensor_reduce(
                    out=dmin[:], in_=dd[:].rearrange("p (c k) -> p c k", k=R),
                    axis=mybir.AxisListType.X, op=ALU.min)
                dminb = bass.AP(dmin[:].tensor, dmin[:].offset,
                                [dmin[:].ap[0], [1, NC_h], [0, R]])
                mask = epi_pool.tile([P, FKh], F32, tag=f"mk{hh}",
                                     name=f"mk{hh}")
                nc.vector.tensor_tensor(
                    out=mask[:].rearrange("p (c k) -> p c k", k=R),
                    in0=dd[:].rearrange("p (c k) -> p c k", k=R),
                    in1=dminb, op=ALU.is_le)
                # first-attaining candidate: k* = min_k (iota_k + R*(1-mask))
                iotap = bass.AP(consts_sb[:].tensor, consts_sb[:].offset + R,
                               [consts_sb[:].ap[0], [0, NC_h], [1, R]])
                tkm = epi_pool.tile([P, FKh], F32, tag=f"tm{hh}",
                                    name=f"tm{hh}")
                nc.vector.tensor_scalar_mul(tkm[:], mask[:], -float(R))
                tk = epi_pool.tile([P, FKh], F32, tag=f"tk{hh}",
                                   name=f"tk{hh}")
                nc.vector.tensor_tensor(
                    out=tk[:].rearrange("p (c k) -> p c k", k=R),
                    in0=tkm[:].rearrange("p (c k) -> p c k", k=R),
                    in1=iotap, op=ALU.add)
                kstar = epi_pool.tile([P, NC_h], F32, tag=f"ks{hh}",
                                      name=f"ks{hh}")
                nc.vector.tensor_reduce(
                    out=kstar[:], in_=tk[:].rearrange("p (c k) -> p c k", k=R),
                    axis=mybir.AxisListType.X, op=ALU.min)
                ksb = bass.AP(kstar[:].tensor, kstar[:].offset,
                              [kstar[:].ap[0], [1, NC_h], [0, R]])
                onehot = epi_pool.tile([P, FKh], F32, tag=f"oh{hh}",
                                       name=f"oh{hh}")
                nc.vector.tensor_tensor(
                    out=onehot[:].rearrange("p (c k) -> p c k", k=R),
                    in0=bass.AP(consts_sb[:].tensor, consts_sb[:].offset,
                                [consts_sb[:].ap[0], [0, NC_h], [1, R]]),
                    in1=ksb, op=ALU.is_equal)
                p5k = epi_pool.tile([P, FKh], F32, tag=f"pk{hh}",
                                    name=f"pk{hh}")
                nc.vector.tensor_reduce(
                    out=p5k[:], in_=p5e[:].rearrange("p (k d) -> p k d", d=3),
                    axis=mybir.AxisListType.X, op=ALU.add,
                    apply_absolute_value=True)
                psel = epi_pool.tile([P, FKh], F32, tag=f"pl{hh}",
                                     name=f"pl{hh}")
                nc.gpsimd.tensor_mul(psel[:], p5k[:], onehot[:])
                col = (dirn - 1) * 5 + half
                nc.vector.reduce_sum(partials[:, col:col + 1], psel[:],
                                     axis=mybir.AxisListType.X)

            def pair_body(dirn, c0):
                """Process chunks c0, c0+1: copies into one buffer, one
                fused fold chain (halves the DVE inter-op overhead)."""
                lhsT_all = aug(0) if dirn == 1 else aug(2)
                rhs_all = aug(1) if dirn == 1 else aug(3)
                tbl = tbl1 if dirn == 1 else tbl2
                cb2 = eb_pool.tile([P, 2 * N], FP16, tag="cb2")
                for cc in range(2):
                    c = c0 + cc
                    for half in range(NH):
                        ps = psum_pool.tile([P, HALF], F32, tag="ps",
                                            space="PSUM")
                        for k in range(HALF // MMFD):
                            m0 = half * HALF + k * MMFD
                            nc.tensor.matmul(
                                ps[:, k * MMFD:(k + 1) * MMFD],
                                lhsT=lhsT_all[:, c * P:(c + 1) * P],
                                rhs=rhs_all[:, m0:m0 + MMFD],
                                start=True, stop=True,
                            )
                        # (DMA cannot read PSUM: ACT does the copies)
                        nc.scalar.activation(
                            out=cb2[:, (cc * NH + half) * HALF:
                                    (cc * NH + half + 1) * HALF],
                            in_=ps[:], func=AF.Copy,
                            bias=0.0, scale=1.0)
                # fused fold-by-halves for BOTH chunks at DVE 2x
                # (per chunk: 4096 -> 256 stride-256 residue groups)
                f1 = fold_pool.tile([P, 4096], FP16, tag="f1")
                nc.vector.tensor_tensor(
                    out=f1[:].rearrange("p (c m) -> p c m", c=2),
                    in0=cb2[:].rearrange("p (c h m) -> p c h m",
                                         c=2, h=2)[:, :, 0, :],
                    in1=cb2[:].rearrange("p (c h m) -> p c h m",
                                         c=2, h=2)[:, :, 1, :],
                    op=ALU.max)
                f2 = fold_pool.tile([P, 2048], FP16, tag="f2")
                nc.vector.tensor_tensor(
                    out=f2[:].rearrange("p (c m) -> p c m", c=2),
                    in0=f1[:].rearrange("p (c h m) -> p c h m",
                                        c=2, h=2)[:, :, 0, :],
                    in1=f1[:].rearrange("p (c h m) -> p c h m",
                                        c=2, h=2)[:, :, 1, :],
                    op=ALU.max)
                f3 = fold_pool.tile([P, 1024], FP16, tag="f3")
                nc.vector.tensor_tensor(
                    out=f3[:].rearrange("p (c m) -> p c m", c=2),
                    in0=f2[:].rearrange("p (c h m) -> p c h m",
                                        c=2, h=2)[:, :, 0, :],
                    in1=f2[:].rearrange("p (c h m) -> p c h m",
                                        c=2, h=2)[:, :, 1, :],
                    op=ALU.max)
                u2 = u_pool.tile([P, 2 * NG], FP16, tag="u2")
                nc.vector.tensor_tensor(
                    out=u2[:].rearrange("p (c m) -> p c m", c=2),
                    in0=f3[:].rearrange("p (c h m) -> p c h m",
                                        c=2, h=2)[:, :, 0, :],
                    in1=f3[:].rearrange("p (c h m) -> p c h m",
                                        c=2, h=2)[:, :, 1, :],
                    op=ALU.max)
                for cc in range(2):
                    c = c0 + cc
                    u_ap = u2[:, cc * NG:(cc + 1) * NG]
                    # winning group id via Max8
                    top8 = idx_pool.tile([P, 8], F32, tag="top8")
                    nc.vector.max(top8[:], u_ap)
                    idx8 = idx_pool.tile([P, 8], U32, tag="idx8")
                    nc.vector.max_index(idx8[:], top8[:], u_ap)
                    nc.gpsimd.indirect_dma_start(
                        out=cand[dirn][:, c * R * 3:(c + 1) * R * 3],
                        out_offset=None,
                        in_=tbl,
                        in_offset=bass.IndirectOffsetOnAxis(
                            ap=idx8[:, 0:1], axis=0),
                    )

            # Interleave the two directions; pairs of chunks per step.
            for c in range(0, NCH, 2):
                for dirn in (1, 2):
                    pair_body(dirn, c)
                # epilogue pieces, emitted after their data is ready
                if c == NCH // 2 + 2:
                    epilogue(1, 0, 0, NCH // 2)
                elif c == NCH // 2 + 4:
                    epilogue(2, 0, 0, NCH // 2)
                elif c == NCH - 2:
                    epilogue(1, 1, NCH // 2, NCH - 1)
                    epilogue(1, 2, NCH - 1, NCH)
                    epilogue(2, 1, NCH // 2, NCH - 1)
                    epilogue(2, 2, NCH - 1, NCH)

            nc.sync.dma_start(out_s, partials[:])

    nc.compile()
    return nc


def _to_bf16(a):
    return a.astype(ml_dtypes.bfloat16)


def _split3(a):
    a = np.asarray(a, np.float32)
    h = _to_bf16(a)
    m = _to_bf16(a - h.astype(np.float32))
    l = _to_bf16(a - h.astype(np.float32) - m.astype(np.float32))
    return h, m, l


def _gather_table(pts):
    """[512, 48] table: rows 0-255 stride-256 residue classes (fold
    chunks), rows 256-511 contiguous 16-groups (direct chunks)."""
    NG = N_FULL // R
    strided = pts.reshape(R, NG, 3).transpose(1, 0, 2)
    contig = pts.reshape(NG, R, 3)
    return np.ascontiguousarray(
        np.concatenate([strided, contig]).reshape(2 * NG, R * 3), np.float32)


def _host_prep(xb, yb):
    xb = np.ascontiguousarray(xb, dtype=np.float32)
    yb = np.ascontiguousarray(yb, dtype=np.float32)
    n = xb.shape[0]
    ones = np.ones((n,), np.float32)

    def build(sta, mov, key_sq, own_sq):
        """bf16 split terms for key = sum_c sta_c*(2 mov_c) - |mov|^2
        - |sta|^2 = -(squared distance), as seen with `sta` stationary;
        key_sq = -(|mov|^2), own_sq = -(|sta|^2)."""
        ta, tb = [], []
        for c in range(3):
            a, b = _split3(sta[:, c]), _split3(2.0 * mov[:, c])
            for i, j in ((0, 0), (0, 1), (0, 2), (1, 0), (1, 1), (2, 0)):
                ta.append(a[i])
                tb.append(b[j])
        sh, sm, sl = _split3(key_sq)
        ob = _to_bf16(ones)
        for s in (sh, sm, sl):
            ta.append(ob)
            tb.append(s)
        oh, om, ol = _split3(own_sq)
        for s in (oh, om, ol):
            ta.append(s)
            tb.append(ob)
        A = np.stack(ta).astype(ml_dtypes.bfloat16)
        Bm = np.stack(tb).astype(ml_dtypes.bfloat16)
        return A, Bm

    y2 = -(yb * yb).sum(-1)
    x2 = -(xb * xb).sum(-1)
    A1, B1 = build(xb, yb, y2, x2)   # dir 1: lhsT = x terms, rhs = y terms
    A2, B2 = build(yb, xb, x2, y2)   # dir 2: lhsT = y terms, rhs = x terms

    augs = np.empty((KSPLIT, 4 * n), ml_dtypes.bfloat16)
    augs[:, 0 * n:1 * n] = A1
    augs[:, 1 * n:2 * n] = B1
    augs[:, 2 * n:3 * n] = A2
    augs[:, 3 * n:4 * n] = B2

    iota = np.arange(R, dtype=np.float32)
    consts = np.tile(np.concatenate([iota, iota + R])[None, :], (P, 1))
    return {"augs": augs, "xr": xb, "yr": yb,
            "tbl1": _gather_table(yb), "tbl2": _gather_table(xb),
            "consts": np.ascontiguousarray(consts, np.float32)}


_NC = None


def _get_nc():
    global _NC
    if _NC is None:
        _NC = _build_nc()
    return _NC


def run_on_hw(x, y, **spmd_kwargs):
    """Run the SPMD kernel; returns (per-core out arrays, BassKernelResults)."""
    x = np.asarray(x, dtype=np.float32)
    y = np.asarray(y, dtype=np.float32)
    assert x.shape == (B, N_FULL, 3) and y.shape == (B, N_FULL, 3)
    nc = _get_nc()
    in_maps = [_host_prep(x[b], y[b]) for b in range(B)]
    res = bass_utils.run_bass_kernel_spmd(
        nc, in_maps, core_ids=list(range(B)), **spmd_kwargs)
    outs = [res.results[b]["out_s"] for b in range(B)]
    return outs, res


def kernel(x, y):
    outs, _ = run_on_hw(x, y)
    vals = []
    for o in outs:
        s = np.asarray(o, dtype=np.float64).sum(axis=0)  # [10] piece-partials
        s1 = s[:5].sum()
        s2 = s[5:].sum()
        vals.append(s1 ** 0.2 + s2 ** 0.2)
    return np.float32(np.mean(vals))
